# revision 43
# baseline (speedup 1.0000x reference)
"""Cross-attention kernel for Trainium2, 8 NeuronCores.

Sharding: core = (batch b in 0..3) x (head-group hg in 0..1).
Each core computes, for its batch and its 8 heads (512 of the 1024 H cols):
    kT = (Wk_h^T @ key_value[b]^T)    [512, SKV]  (+bk)      bf16
    v  = key_value[b] @ Wv_h          per kv-tile [128, 8*65] fp16
                                      (ones column per head = softmax denom)
    qT = (Wq_h^T @ query[b]^T)        [512, SQ]   (+bq)      bf16
    attention per (q-chunk 512, head-pair, kv-tile):
        scores for the two heads of a pair -> two [128,512] matmuls with
        K=64 at partition bases 0/64: TensorE row-tiling (tiles T0/T8) lets
        them stream concurrently in the two array halves
        exp: one ACTIVATE over the adjacent PSUM quarters -> et [128,1024]
        attnV: per head [65,512] PSUM accum over kv tiles (ones col = denom)
        normalize: DVE drain, recip, GPSIMD partition_broadcast, DVE mul
    out-proj: outT^T-slices @ Wo_h -> [SQ, 1024] bf16 partials, DMA out.
Host sums the two head-group partials per batch and adds bv@Wo + bo.

Performance design: the PE must never idle (the HAM activity monitor
halves the PE clock 2.4->1.2GHz on micro-gaps; the old kernel ran 668us
entirely at 1.2GHz), so everything except a tiny pre-phase is one
continuous PE stream: per-slot scores+attnV with all projection/out-proj
work interleaved as filler between slots, earliest-deadline order.  attnV
lags exp by LAG slots through a deep fp16 et pool.  A head-pair's two
K=64 score matmuls sit at partition bases 0/64, so the PE executes them
CONCURRENTLY in the two row-tile halves (T0/T8) of the array (~2x on
scores; hw-verified +3ns issue distance).  Each pair's scores share one
[128,1024] PSUM pool tile (2 bufs) so one ACTIVATE covers both heads; a
manually-rotated single big PSUM tile false-shares in the dependency
tracker and serializes sc behind exp (costs ~150us) - keep the pool.
ACT (exp, ~1.1us per [128,1024] instr, the only engine with exp) runs
~274us underneath the ~340us PE stream.  Startup DMA is split across the
sync+gpsimd queues (descriptor gen ~550ns each is the lead-in limiter).
Measured: 378us vs 668us baseline, rel err 4.1e-3 (gate 2e-2).
"""

from collections import deque

import ml_dtypes
import numpy as np

import concourse.bass as bass
import concourse.mybir as mybir
import concourse.tile as tile
from concourse import bacc
from concourse import bass_utils

FP32 = mybir.dt.float32
FP32R = mybir.dt.float32r
BF16 = mybir.dt.bfloat16
F16 = mybir.dt.float16
P = 128

B, SQ, SKV = 4, 2048, 2048
D, H, NH, HD = 1024, 1024, 16, 64
HC = 512          # H columns per core (8 heads)
NHC = 8           # heads per core
NHP = 4           # head pairs per core
VW = HD + 1       # v columns per head incl. ones column

ND = D // P       # 8 contraction chunks for projections
NI = HC // P      # 4 Hc tiles
NKT = SKV // P    # 16 kv tiles
QC = 512          # attention q chunk
NQC = SQ // QC    # 4
SUB = 512         # matmul moving-operand max
NCK = 4           # kv/q chunks of 512 for x streaming
CK = 512
LAG = 3           # attnV pair-slots behind exp


def build_core_program(n_devices=8):
    nc = bacc.Bacc(
        "TRN2",
        target_bir_lowering=False,
        debug=False,
        enable_asserts=False,
        num_devices=n_devices,
    )

    xqT = nc.dram_tensor("xqT", (D, SQ), BF16, kind="ExternalInput").ap()
    xkT = nc.dram_tensor("xkT", (D, SKV), BF16, kind="ExternalInput").ap()
    wq = nc.dram_tensor("wq", (D, HC), BF16, kind="ExternalInput").ap()
    wk = nc.dram_tensor("wk", (D, HC), BF16, kind="ExternalInput").ap()
    wv = nc.dram_tensor("wv", (D, HC), BF16, kind="ExternalInput").ap()
    wo = nc.dram_tensor("wo", (HC, D), BF16, kind="ExternalInput").ap()
    bq = nc.dram_tensor("bq", (HC, 1), FP32, kind="ExternalInput").ap()
    bk = nc.dram_tensor("bk", (HC, 1), FP32, kind="ExternalInput").ap()
    out = nc.dram_tensor("out", (SQ, D), BF16, kind="ExternalOutput").ap()

    EXP = mybir.ActivationFunctionType.Exp

    with nc.allow_low_precision(reason="bf16/fp16 attention pipeline"), tile.TileContext(nc) as tc:
        with (
            tc.tile_pool(name="persist", bufs=1) as persist,
            tc.tile_pool(name="xq", bufs=16) as xqp,
            tc.tile_pool(name="et", bufs=12) as etp,
            tc.tile_pool(name="ovsb", bufs=4) as ovsbp,
            tc.tile_pool(name="recp", bufs=4) as recp,
            tc.tile_pool(name="bcsp", bufs=4) as bcsp,
            tc.tile_pool(name="otp", bufs=1) as otp,
            tc.tile_pool(name="ost", bufs=4) as ostp,
            tc.tile_pool(name="scps", bufs=2, space=bass.MemorySpace.PSUM) as scps,
            tc.tile_pool(name="ovps", bufs=1, space=bass.MemorySpace.PSUM) as ovps,
            tc.tile_pool(name="fps", bufs=2, space=bass.MemorySpace.PSUM) as fps,
        ):
            # ---------------- persistent tiles ----------------
            kT = [persist.tile([P, SKV], BF16, tag=f"kT{i}", name=f"kT{i}") for i in range(NI)]
            qTc = [
                [persist.tile([P, QC], BF16, tag=f"qT{c}_{i}", name=f"qT{c}_{i}") for i in range(NI)]
                for c in range(NQC)
            ]
            vaug = [persist.tile([P, NHC * VW], F16, tag=f"v{t}", name=f"v{t}") for t in range(NKT)]
            xk = [
                [persist.tile([P, CK], BF16, tag=f"xk{ck}_{d}", name=f"xk{ck}_{d}") for d in range(ND)]
                for ck in range(NCK)
            ]
            wq_sb = [persist.tile([P, HC], BF16, tag=f"wq{d}", name=f"wq{d}") for d in range(ND)]
            wk_sb = [persist.tile([P, HC], BF16, tag=f"wk{d}", name=f"wk{d}") for d in range(ND)]
            wv_sb = [persist.tile([P, HC], BF16, tag=f"wv{d}", name=f"wv{d}") for d in range(ND)]
            wo_sb = [persist.tile([P, D], BF16, tag=f"wo{j}", name=f"wo{j}") for j in range(NI)]
            bqs = persist.tile([P, NI], FP32, tag="bqs")
            bks = persist.tile([P, NI], FP32, tag="bks")
            warm = persist.tile([1, 8], FP32, tag="warm")

            ove = ovps.tile([VW, QC], FP32, tag="ove", name="ove", bufs=1)
            ovo = ovps.tile([VW, QC], FP32, tag="ovo", name="ovo", bufs=1)

            # preload the exp activation table during the DMA lead-in
            nc.vector.memset(warm[:], 0.0)
            nc.scalar.activation(warm[:], warm[:], EXP)

            # ---------------- DMA issue (first-use order, 2 queues) -------
            # descriptor generation is ~550ns per DMA per queue, which is
            # the startup bottleneck; split across sync + gpsimd queues
            for i in range(NI):
                nc.sync.dma_start(out=bqs[:, i : i + 1], in_=bq[i * P : (i + 1) * P, :])
                nc.sync.dma_start(out=bks[:, i : i + 1], in_=bk[i * P : (i + 1) * P, :])
            xq_c0 = []
            for d in range(ND):
                nc.sync.dma_start(out=wk_sb[d][:], in_=wk[d * P : (d + 1) * P, :])
                nc.sync.dma_start(out=xk[0][d][:], in_=xkT[d * P : (d + 1) * P, 0:CK])
                nc.gpsimd.dma_start(out=wq_sb[d][:], in_=wq[d * P : (d + 1) * P, :])
                t = xqp.tile([P, CK], BF16, tag="xq", name="xq")
                nc.gpsimd.dma_start(out=t[:], in_=xqT[d * P : (d + 1) * P, 0:CK])
                xq_c0.append(t)
            for d in range(ND):
                nc.gpsimd.dma_start(out=wv_sb[d][:], in_=wv[d * P : (d + 1) * P, :])
                nc.sync.dma_start(
                    out=xk[1][d][:], in_=xkT[d * P : (d + 1) * P, CK : 2 * CK]
                )
            for ck in range(2, NCK):
                for d in range(ND):
                    nc.sync.dma_start(
                        out=xk[ck][d][:], in_=xkT[d * P : (d + 1) * P, ck * CK : (ck + 1) * CK]
                    )
            for j in range(NI):
                nc.gpsimd.dma_start(out=wo_sb[j][:], in_=wo[j * P : (j + 1) * P, :])
            xq_c1 = []
            for d in range(ND):
                t = xqp.tile([P, CK], BF16, tag="xq", name="xq")
                nc.gpsimd.dma_start(out=t[:], in_=xqT[d * P : (d + 1) * P, CK : 2 * CK])
                xq_c1.append(t)
            xq_chunks = {0: xq_c0, 1: xq_c1}

            # vaug ones columns (persist; v-proj writes leave them alone)
            for t in range(NKT):
                vv = vaug[t].rearrange("p (h w) -> p h w", w=VW)
                nc.gpsimd.memset(vv[:, :, HD : HD + 1], 1.0)

            # ---------------- unit thunks (filler PE work) ----------------
            def kT_unit(i, ck):
                ps = fps.tile([P, SUB], FP32, tag="fp", name="fp")
                for d in range(ND):
                    nc.tensor.matmul(
                        ps[:],
                        wk_sb[d][:, i * P : (i + 1) * P],
                        xk[ck][d][:],
                        start=(d == 0),
                        stop=(d == ND - 1),
                    )
                nc.vector.tensor_scalar_add(
                    out=kT[i][:, ck * CK : (ck + 1) * CK],
                    in0=ps[:],
                    scalar1=bks[:, i : i + 1],
                )

            def v_unit(t):
                ck, tt = t // 4, t % 4
                ps = fps.tile([P, SUB], FP32, tag="fp", name="fp")
                for d in range(ND):
                    nc.tensor.matmul(
                        ps[:],
                        xk[ck][d][:, tt * P : (tt + 1) * P],
                        wv_sb[d][:],
                        start=(d == 0),
                        stop=(d == ND - 1),
                    )
                vv = vaug[t].rearrange("p (h w) -> p h w", w=VW)
                nc.vector.tensor_copy(
                    out=vv[:, :, 0:HD],
                    in_=ps.rearrange("p (h w) -> p h w", w=HD),
                )

            def load_xq_chunk(cq):
                tiles = []
                for d in range(ND):
                    t = xqp.tile([P, CK], BF16, tag="xq", name="xq")
                    nc.sync.dma_start(
                        out=t[:], in_=xqT[d * P : (d + 1) * P, cq * CK : (cq + 1) * CK]
                    )
                    tiles.append(t)
                xq_chunks[cq] = tiles

            def qp_unit(c, i):
                ps = fps.tile([P, SUB], FP32, tag="fp", name="fp")
                for d in range(ND):
                    nc.tensor.matmul(
                        ps[:],
                        wq_sb[d][:, i * P : (i + 1) * P],
                        xq_chunks[c][d][:],
                        start=(d == 0),
                        stop=(d == ND - 1),
                    )
                nc.vector.tensor_scalar_add(
                    out=qTc[c][i][:], in0=ps[:], scalar1=bqs[:, i : i + 1]
                )

            def op_unit(c, m, n):
                # out-proj: out[q-tile m of chunk c, 512-col n] = sum_j outT^T @ wo
                ps = fps.tile([P, SUB], FP32, tag="fp", name="op")
                for j in range(NI):
                    nc.tensor.matmul(
                        ps[:],
                        outT_of[c][j][:, m * P : (m + 1) * P],
                        wo_sb[j][:, n * SUB : (n + 1) * SUB],
                        start=(j == 0),
                        stop=(j == NI - 1),
                    )
                ot = ostp.tile([P, SUB], BF16, tag="ot", name="ot")
                nc.vector.tensor_copy(out=ot[:], in_=ps[:])
                qm = c * (QC // P) + m
                nc.sync.dma_start(
                    out=out[qm * P : (qm + 1) * P, n * SUB : (n + 1) * SUB], in_=ot[:]
                )

            # ---------------- pre phase ----------------
            kT_unit(0, 0)
            qp_unit(0, 0)
            v_emit_slot = {}

            # ---------------- filler list (earliest-deadline order) -------
            # deadlines (pair-slot): kT[i]-ck -> slot 16i+4ck; v[t] -> av-lag
            # tolerance ~slot t+6 (et-pool depth); qp(c,i) -> slot 64c+16i.
            fillers = deque(
                [
                    ("v", 0), ("v", 1), ("kT", 0, 1), ("kT", 0, 2),
                    ("v", 2), ("v", 3), ("kT", 0, 3), ("v", 4), ("v", 5),
                    ("kT", 1, 0), ("qp", 0, 1), ("v", 6), ("v", 7),
                    ("v", 8), ("v", 9), ("kT", 1, 1), ("v", 10), ("v", 11),
                    ("v", 12), ("v", 13), ("kT", 1, 2), ("v", 14), ("v", 15),
                    ("kT", 1, 3), ("kT", 2, 0), ("qp", 0, 2), ("kT", 2, 1),
                    ("kT", 2, 2), ("kT", 2, 3), ("kT", 3, 0), ("qp", 0, 3),
                    ("kT", 3, 1), ("kT", 3, 2), ("kT", 3, 3),
                    ("qp", 1, 0), ("qp", 1, 1), ("qp", 1, 2), ("qp", 1, 3),
                ]
            )
            # late work is deadline-timed instead of front-loaded, so the
            # back half of the stream keeps the PE fed (HAM stays warm)
            timed = deque(
                [
                    (88, ("xq", 2)),
                    (92, ("qp", 2, 0)),
                    (98, ("qp", 2, 1)),
                    (104, ("qp", 2, 2)),
                    (110, ("qp", 2, 3)),
                    (148, ("xq", 3)),
                    (156, ("qp", 3, 0)),
                    (162, ("qp", 3, 1)),
                    (168, ("qp", 3, 2)),
                    (174, ("qp", 3, 3)),
                ]
            )

            def run_filler(f, slot):
                kind = f[0]
                if kind == "kT":
                    kT_unit(f[1], f[2])
                elif kind == "v":
                    v_unit(f[1])
                    v_emit_slot[f[1]] = slot
                elif kind == "qp":
                    qp_unit(f[1], f[2])
                elif kind == "xq":
                    load_xq_chunk(f[1])
                elif kind == "op":
                    op_unit(f[1], f[2], f[3])

            # ---------------- attention slot stream ----------------
            outT_of = {}
            et_of = {}
            sc_slot = {}
            drain_slot = {}
            ovsb_of = {}
            post = []  # (due_slot, c, hp)
            av_q = deque()
            slot = 0

            def emit_sc_exp(c, hp, t, slot):
                # one [128, 1024] PSUM pool tile per kv-tile holds BOTH heads'
                # scores side by side: the two K=64 matmuls land on PE row
                # tiles T0/T8 (concurrent), one ACTIVATE covers the pair
                scp = scps.tile([P, 2 * QC], FP32, tag="scp", name="scp")
                for r, q in ((0, 0), (HD, 1)):
                    nc.tensor.matmul(
                        scp[:, q * QC : (q + 1) * QC],
                        kT[hp][r : r + HD, t * P : (t + 1) * P],
                        qTc[c][hp][r : r + HD, :],
                        start=True,
                        stop=True,
                    )
                et = etp.tile([P, 2 * QC], F16, tag="et", name="et")
                nc.scalar.activation(et[:], scp[:], EXP, scale=0.125)
                et_of[(c, hp, t)] = et
                sc_slot[(c, hp, t)] = slot

            def emit_av(c, hp, t):
                et = et_of.pop((c, hp, t))
                for ovt, h, eo in ((ove, 2 * hp, 0), (ovo, 2 * hp + 1, 1)):
                    nc.tensor.matmul(
                        ovt[:],
                        vaug[t][:, h * VW : (h + 1) * VW],
                        et[:, eo * QC : (eo + 1) * QC],
                        start=(t == 0),
                        stop=(t == NKT - 1),
                    )

            def emit_drain(c, hp):
                for ovt, eo in ((ove, 0), (ovo, 1)):
                    ovsb = ovsbp.tile([HD, QC], FP32, tag="ovsb", name="ovsb")
                    rec = recp.tile([1, QC], FP32, tag="rec", name="rec")
                    nc.vector.tensor_copy(out=ovsb[:], in_=ovt[0:HD, :])
                    nc.vector.tensor_copy(out=rec[:], in_=ovt[HD : HD + 1, :])
                    ovsb_of[(c, 2 * hp + eo)] = (ovsb, rec)

            def emit_norm(c, hp):
                for eo in (0, 1):
                    h = 2 * hp + eo
                    i, r = hp, eo * HD
                    ovsb, rec = ovsb_of.pop((c, h))
                    nc.vector.reciprocal_approx_fast(out=rec[:], in_=rec[:])
                    bcs = bcsp.tile([HD, QC], FP32, tag="bcs", name="bcs")
                    nc.gpsimd.partition_broadcast(bcs[:], rec[:])
                    nc.vector.tensor_mul(
                        out=outT_of[c][i][r : r + HD, :], in0=ovsb[:], in1=bcs[:]
                    )
                if hp == NHP - 1:
                    # chunk fully normalized -> out-proj spread over the
                    # next chunk's slots as timed filler
                    base = slot + 2
                    for k, (m, n) in enumerate(
                        (m, n) for m in range(QC // P) for n in range(D // SUB)
                    ):
                        timed.append((base + 6 * k, ("op", c, m, n)))

            def av_poppable(slot):
                if not av_q:
                    return False
                c, hp, t = av_q[0]
                if slot - sc_slot[(c, hp, t)] < LAG:
                    return False
                if c == 0 and slot - v_emit_slot.get(t, 10**9) < 2:
                    return False
                if t == 0:
                    prev = (c, hp - 1) if hp > 0 else (c - 1, NHP - 1)
                    if prev[0] >= 0:
                        ds = drain_slot.get(prev)
                        if ds is None or slot - ds < 2:
                            return False
                return True

            def pop_avs(slot, maxn=2):
                n = 0
                while n < maxn and av_poppable(slot):
                    c, hp, t = av_q.popleft()
                    emit_av(c, hp, t)
                    n += 1
                    if t == NKT - 1:
                        emit_drain(c, hp)
                        drain_slot[(c, hp)] = slot
                        post.append((slot + 1, c, hp))

            def run_due_posts(slot):
                while post and post[0][0] <= slot:
                    _, c, hp = post.pop(0)
                    emit_norm(c, hp)

            for c in range(NQC):
                outT_of[c] = [
                    otp.tile([P, QC], BF16, tag=f"oT{j}", name=f"oT{j}", bufs=2)
                    for j in range(NI)
                ]
                for hp in range(NHP):
                    # two pair-slots per iteration: both sc pairs emitted in
                    # one 64-row-mode block, halving 64<->128 mode switches
                    for t in range(0, NKT, 2):
                        run_due_posts(slot)
                        emit_sc_exp(c, hp, t, slot)
                        emit_sc_exp(c, hp, t + 1, slot + 1)
                        pop_avs(slot, maxn=6)
                        if fillers and slot < 24:
                            run_filler(fillers.popleft(), slot)
                        if fillers:
                            run_filler(fillers.popleft(), slot)
                        elif timed and timed[0][0] <= slot:
                            run_filler(timed.popleft()[1], slot)
                        pop_avs(slot, maxn=2)
                        av_q.append((c, hp, t))
                        av_q.append((c, hp, t + 1))
                        slot += 2

            # ---------------- tail ----------------
            guard = 0
            while av_q or post:
                run_due_posts(slot)
                pop_avs(slot, maxn=6)
                if timed:
                    run_filler(timed.popleft()[1], slot)
                if timed:
                    run_filler(timed.popleft()[1], slot)
                slot += 1
                guard += 1
                assert guard < 500, "tail drain stuck"
            while fillers or timed:
                if fillers:
                    run_filler(fillers.popleft(), slot)
                else:
                    run_filler(timed.popleft()[1], slot)
                slot += 1

    nc.compile()
    return nc


_CACHED_NC = None


def _get_nc():
    global _CACHED_NC
    if _CACHED_NC is None:
        _CACHED_NC = build_core_program()
    return _CACHED_NC


def make_in_maps(query, key_value, Wq, bq, Wk, bk, Wv, bv, Wo, bo):
    query = np.asarray(query, np.float32)
    key_value = np.asarray(key_value, np.float32)
    Wq = np.asarray(Wq, np.float32)
    Wk = np.asarray(Wk, np.float32)
    Wv = np.asarray(Wv, np.float32)
    Wo = np.asarray(Wo, np.float32)
    bq = np.asarray(bq, np.float32)
    bk = np.asarray(bk, np.float32)
    bf = ml_dtypes.bfloat16

    in_maps = []
    for core in range(8):
        b, hg = core // 2, core % 2
        hs = hg * HC
        in_maps.append(
            {
                "xqT": np.ascontiguousarray(query[b].T).astype(bf),
                "xkT": np.ascontiguousarray(key_value[b].T).astype(bf),
                "wq": np.ascontiguousarray(Wq[:, hs : hs + HC]).astype(bf),
                "wk": np.ascontiguousarray(Wk[:, hs : hs + HC]).astype(bf),
                "wv": np.ascontiguousarray(Wv[:, hs : hs + HC]).astype(bf),
                "wo": np.ascontiguousarray(Wo[hs : hs + HC, :]).astype(bf),
                "bq": np.ascontiguousarray(bq[hs : hs + HC, None]),
                "bk": np.ascontiguousarray(bk[hs : hs + HC, None]),
            }
        )
    return in_maps


def _install_profiling():
    """Reconstruct the NTFF profile hook this container's boot skipped.

    bass_utils' axon trace path wants antenv.axon_hooks (absent here);
    inject a stub module and register the ctypes-based hook from
    trn_agent_boot. Also keep artifacts local (no bucket in container).
    """
    import sys
    import types

    if "antenv.axon_hooks" in sys.modules:
        return
    import antenv  # noqa: F401

    mod = types.ModuleType("antenv.axon_hooks")
    mod._hook = None

    def set_axon_ntff_profile_hook(h):
        mod._hook = h

    def get_axon_ntff_profile_hook():
        return mod._hook

    mod.set_axon_ntff_profile_hook = set_axon_ntff_profile_hook
    mod.get_axon_ntff_profile_hook = get_axon_ntff_profile_hook
    sys.modules["antenv.axon_hooks"] = mod

    from trn_agent_boot.trn_boot import _ntff_profile_via_ctypes

    hook = _ntff_profile_via_ctypes("/opt/axon/libaxon_pjrt.so")
    if hook is not None:
        set_axon_ntff_profile_hook(hook)

    bass_utils.upload_artifacts = lambda tmpdir: tmpdir


def run_device(inputs, trace=False, **kw):
    if trace:
        _install_profiling()
    nc = _get_nc()
    in_maps = make_in_maps(**inputs)
    res = bass_utils.run_bass_kernel_spmd(
        nc, in_maps, list(range(8)), trace=trace, **kw
    )
    return res


def assemble_output(results, Wv_bias_term):
    out = np.zeros((B, SQ, D), np.float32)
    for core in range(8):
        b = core // 2
        out[b] += np.asarray(results[core]["out"], dtype=np.float32)
    out += Wv_bias_term
    return out


def kernel(**inputs):
    res = run_device(inputs)
    bv = np.asarray(inputs["bv"], np.float32)
    bo = np.asarray(inputs["bo"], np.float32)
    Wo = np.asarray(inputs["Wo"], np.float32)
    # attn rows sum to 1, so the bv shift passes straight through attn@v;
    # bv@Wo + bo is added once on the host.
    bias_term = bv @ Wo + bo
    return assemble_output(res.results, bias_term)
